# revision 21
# baseline (speedup 1.0000x reference)
"""Trainium2 Bass kernel for GNN message passing (IntraConv + BatchNorm).

Computation (reference):
    msg   = feat[src] * edge_weight                    [E, D]
    neigh = segment_sum(msg, dst, N)                   [N, D]
    deg   = segment_sum(edge_weight, dst, N)           [N, 1]
    h     = relu(feat @ Ws.T + b_self + (neigh/(deg+eps)) @ Wn.T + bias)
    out   = batchnorm(h; gamma, beta)  (training-mode batch stats)

Distribution over 8 NeuronCores: edges sorted by dst and sharded by dst-range
so each core owns N/8 contiguous nodes and every edge pointing at them.
Local segment sums are exact; the only collective is an AllReduce of the
[128, 2] BatchNorm statistics.

Per-core pipeline (v7, ~3.1x over the first working version):
  - bf16 feature table (256B rows) split at 32768 for int16 dma_gather
    indices; per-128-dst-tile gathers are issued round-robin over 4
    SWDGE queues so all four Q7 core-pairs generate descriptors
    concurrently (~3.5x gather throughput; the gather is descriptor-
    generation-bound at ~2.8ns/index aggregate, not HBM-bound).
  - per-tile chunk counts (max over cores, one SPMD program) instead of
    a global max: 8.6% fewer padded gather slots.  num_idxs register
    values are hoisted into persistent registers.
  - one-hot-times-weight S_w[e,d] = (d == dstl[e]) * w~[e] built with two
    pair-trick tensor_tensor ops in bf16 2x DVE mode, where w~ folds the
    host-precomputed degree normalization 1/(deg[dst]+eps) (pure graph
    preprocessing) into the edge weight.
  - PE computes sum_e g[e,f] * S_w[e,d] with g as the stationary operand,
    so PSUM holds h_neigh.T (feature-major) directly - no transpose, no
    separate degree matmuls; ACT evacuates PSUM to the hnT slab.
  - linear chunks (512 nodes) interleave with the tile loop as their hnT
    columns complete; bias+relu and BN partial stats on ACT (Relu /
    Square with accum_out); one [128, 2] AllReduce; scale/shift; output
    written feature-major [128, N/8] and transposed on the host.
"""

import numpy as np
import ml_dtypes
from contextlib import ExitStack

import concourse.bass as bass
import concourse.tile as tile
from concourse import bacc, mybir
from concourse.bass_utils import run_bass_kernel_spmd
from concourse.masks import make_identity

N_CORES = 8
P = 128
HALF = 32768        # int16 index limit for dma_gather
LIN_CHUNK = 512
GMAX = 8            # dma_gather is limited to 1024 indices per instruction
EPS_DEG = 1e-8
EPS_BN = 1e-5

NQ = 4              # SWDGE queues: 4 Q7 core-pairs generate descriptors in parallel

F32 = mybir.dt.float32
BF16 = mybir.dt.bfloat16
I16 = mybir.dt.int16
OP = mybir.AluOpType
ACT = mybir.ActivationFunctionType


def _host_plan(feat, src, dst, edge_weight):
    N, D = feat.shape
    E = src.shape[0]
    assert D == P and N % N_CORES == 0
    npc = N // N_CORES                      # nodes per core
    T = (npc + P - 1) // P                  # dst tiles per core
    nw = T * P                              # padded node-slab width
    n_hi = N - HALF if N > HALF else 0

    src64 = src.astype(np.int64)
    dst64 = dst.astype(np.int64)
    ws_all = edge_weight.reshape(-1).astype(np.float32)

    # fold degree normalization into the edge weights (host graph prep):
    # wd_e = w_e / (deg[dst_e] + eps)
    deg_all = np.bincount(dst64, weights=ws_all.astype(np.float64), minlength=N)
    dinv_all = (1.0 / (deg_all + EPS_DEG)).astype(np.float32)
    ws_all = ws_all * dinv_all[dst64]

    half = (src64 >= HALF).astype(np.int64)
    ct = (dst64 // npc) * T + (dst64 % npc) // P      # (core, tile) group id
    order = np.lexsort((half, ct))
    ss = src64[order]
    ws = ws_all[order]
    hh = half[order]
    cts = ct[order]
    dstl = ((dst64[order] % npc) % P).astype(np.float32)

    grp = cts * 2 + hh                                 # (core, tile, half)
    counts = np.bincount(grp, minlength=N_CORES * T * 2)
    cnt_lo = counts[0::2].reshape(N_CORES, T)
    cnt_hi = counts[1::2].reshape(N_CORES, T)
    # per-tile chunk counts: max over cores (one SPMD program, 8.6% fewer
    # padded slots than a single global max)
    klo_t = np.maximum(np.ceil(cnt_lo.max(axis=0) / P).astype(np.int64), 1)
    if n_hi > 0:
        khi_t = np.maximum(np.ceil(cnt_hi.max(axis=0) / P).astype(np.int64), 1)
    else:
        khi_t = np.zeros(T, np.int64)
    k_t = klo_t + khi_t
    offk = np.zeros(T + 1, np.int64)
    np.cumsum(k_t, out=offk[1:])
    sumK = int(offk[-1])

    starts = np.zeros(N_CORES * T * 2 + 1, np.int64)
    np.cumsum(counts, out=starts[1:])
    pos = np.arange(E, dtype=np.int64) - starts[grp]
    # slot position within the core stream: tile base + (hi? lo-extent) + pos
    tix = cts % T
    q = offk[tix] * P + hh * (klo_t[tix] * P) + pos
    flat = (cts // T) * (sumK * P) + q

    # 0 index padding (the -1 trim path crashes on this runtime);
    # dstl padding 200 never matches iota (0..127); w padding 0.
    idx_stream = np.zeros(N_CORES * sumK * P, np.int32)
    w_stream = np.zeros(N_CORES * sumK * P, np.float32)
    dstl_stream = np.full(N_CORES * sumK * P, 200.0, np.float32)
    idx_stream[flat] = ss - hh * HALF
    w_stream[flat] = ws
    dstl_stream[flat] = dstl

    # stream position q = c*128 + p -> SBUF [P, sumK] at column offk[t] + c
    def to_sb(a):
        return np.ascontiguousarray(
            a.reshape(N_CORES, sumK, P).transpose(0, 2, 1)
        )

    w_sb = np.repeat(to_sb(w_stream).astype(ml_dtypes.bfloat16), 2, axis=2)
    dstl_sb = np.repeat(to_sb(dstl_stream).astype(ml_dtypes.bfloat16), 2, axis=2)

    # gather indices: [16-wrap, replicate x8] per (tile, half)
    ist = idx_stream.reshape(N_CORES, sumK, P)
    idx_cols = []
    for t in range(T):
        for h0, hn in ((offk[t], klo_t[t]), (offk[t] + klo_t[t], khi_t[t])):
            blk = ist[:, h0:h0 + hn].reshape(N_CORES, hn * P)
            blk = blk.reshape(N_CORES, hn * P // 16, 16).transpose(0, 2, 1)
            idx_cols.append(np.tile(blk, (1, 8, 1)))
    idx_sb = np.ascontiguousarray(np.concatenate(idx_cols, axis=2)).astype(np.int16)

    # bf16 gather tables (256B rows)
    feat16 = feat.astype(ml_dtypes.bfloat16)
    feat_lo = np.ascontiguousarray(feat16[:HALF])
    feat_hi = (
        np.ascontiguousarray(feat16[HALF:])
        if n_hi > 0 else np.zeros((1, P), ml_dtypes.bfloat16)
    )

    # per-core self-feature slab, bf16, zero padded to nw rows
    feat_self = np.zeros((N_CORES, nw, P), ml_dtypes.bfloat16)
    fb = feat16.reshape(N_CORES, npc, P)
    for c in range(N_CORES):
        feat_self[c, :npc] = fb[c]

    iota = np.broadcast_to(np.arange(P, dtype=np.float32), (P, P)).astype(
        ml_dtypes.bfloat16
    )

    return dict(
        N=N, E=E, npc=npc, T=T, nw=nw,
        klo_t=tuple(int(x) for x in klo_t),
        khi_t=tuple(int(x) for x in khi_t),
        sumK=sumK,
        n_lo=min(N, HALF), n_hi=max(n_hi, 1),
        idx_sb=idx_sb, w_sb=w_sb, dstl_sb=dstl_sb,
        feat_lo=feat_lo, feat_hi=feat_hi,
        feat_self=feat_self, iota=np.ascontiguousarray(iota),
    )


def _build_program(N, T, klo_t, khi_t, npc, nw, n_lo, n_hi, n_cores=N_CORES):
    k_t = [a + b for a, b in zip(klo_t, khi_t)]
    offk = [0]
    for k in k_t:
        offk.append(offk[-1] + k)
    sumK = offk[-1]
    KMAX = max(k_t)
    nc = bacc.Bacc(
        "TRN2",
        target_bir_lowering=False,
        debug=False,
        enable_asserts=False,
        num_devices=n_cores,
        num_swdge_queues=NQ,
    )

    flo_d = nc.dram_tensor("feat_lo", [n_lo, P], BF16, kind="ExternalInput")
    fhi_d = nc.dram_tensor("feat_hi", [n_hi, P], BF16, kind="ExternalInput")
    idx_d = nc.dram_tensor("idx_sb", [P, sumK * 8], I16, kind="ExternalInput")
    w_d = nc.dram_tensor("w_sb", [P, 2 * sumK], BF16, kind="ExternalInput")
    dstl_d = nc.dram_tensor("dstl_sb", [P, 2 * sumK], BF16, kind="ExternalInput")
    fself_d = nc.dram_tensor("feat_self", [nw, P], BF16, kind="ExternalInput")
    iota_d = nc.dram_tensor("iota", [P, P], BF16, kind="ExternalInput")
    wn_d = nc.dram_tensor("wn_t", [P, P], BF16, kind="ExternalInput")
    ws_d = nc.dram_tensor("ws_t", [P, P], BF16, kind="ExternalInput")
    bias_d = nc.dram_tensor("bias_sum", [P, 1], F32, kind="ExternalInput")
    gamma_d = nc.dram_tensor("gamma_c", [P, 1], F32, kind="ExternalInput")
    beta_d = nc.dram_tensor("beta_c", [P, 1], F32, kind="ExternalInput")

    out_d = nc.dram_tensor("outT", [P, npc], F32, kind="ExternalOutput")

    cc_in = nc.dram_tensor("cc_in", [P, 2], F32)
    cc_out = nc.dram_tensor("cc_out", [P, 2], F32, addr_space="Shared")

    with tile.TileContext(nc) as tc, ExitStack() as ctx:
        const = ctx.enter_context(tc.tile_pool(name="const", bufs=1))
        slabs = ctx.enter_context(tc.tile_pool(name="slabs", bufs=1))
        gpool = ctx.enter_context(tc.tile_pool(name="gpool", bufs=12))
        spool = ctx.enter_context(tc.tile_pool(name="spool", bufs=3))
        swpool = ctx.enter_context(tc.tile_pool(name="swpool", bufs=3))
        small = ctx.enter_context(tc.tile_pool(name="small", bufs=6))
        stage = ctx.enter_context(tc.tile_pool(name="stage", bufs=3))
        ps_acc = ctx.enter_context(tc.tile_pool(name="ps_acc", bufs=2, space="PSUM"))
        ps_lin = ctx.enter_context(tc.tile_pool(name="ps_lin", bufs=2, space="PSUM"))

        # ---- constants: gather-critical streams first ----
        idx_t = const.tile([P, sumK * 8], I16)
        idx_head = (offk[2] if T > 2 else sumK) * 8
        nc.sync.dma_start(idx_t[:, 0:idx_head], idx_d[:, 0:idx_head])
        nc.sync.dma_start(idx_t[:, idx_head:], idx_d[:, idx_head:])
        w_t = const.tile([P, 2 * sumK], BF16)
        nc.sync.dma_start(w_t[:], w_d[:, :])
        dstl_t = const.tile([P, 2 * sumK], BF16)
        nc.sync.dma_start(dstl_t[:], dstl_d[:, :])
        iota_t = const.tile([P, P], BF16)
        nc.sync.dma_start(iota_t[:], iota_d[:, :])
        wn_t = const.tile([P, P], BF16)
        ws_t = const.tile([P, P], BF16)
        bias_t = const.tile([P, 1], F32)
        gamma_t = const.tile([P, 1], F32)
        beta_t = const.tile([P, 1], F32)

        featT = slabs.tile([P, nw], BF16)
        rst = slabs.tile([P, nw], F32)
        hnT = slabs.tile([P, nw], BF16)

        def late_consts():
            # issued after the first tile's gathers so the gather pipeline
            # starts as early as possible
            nc.sync.dma_start(wn_t[:], wn_d[:, :])
            nc.sync.dma_start(ws_t[:], ws_d[:, :])
            nc.sync.dma_start(bias_t[:], bias_d[:, :])
            nc.sync.dma_start(gamma_t[:], gamma_d[:, :])
            nc.sync.dma_start(beta_t[:], beta_d[:, :])
            nc.sync.dma_start_transpose(featT[:], fself_d[:, :])



        def _pair_ap(tile2d, t):
            kk = k_t[t]
            a = tile2d[:, 2 * offk[t]: 2 * offk[t] + 2 * kk]
            pdim = list(a.ap)[0]
            return bass.AP(tensor=a.tensor, offset=a.offset,
                           ap=[pdim, [2, kk], [0, 64], [1, 2]])

        ia = iota_t[:]

        def _iota_rep(t):
            return bass.AP(tensor=ia.tensor, offset=ia.offset,
                           ap=[list(ia.ap)[0], [0, k_t[t]], [1, P]])

        nchunks = (nw + LIN_CHUNK - 1) // LIN_CHUNK
        sum_parts = small.tile([P, nchunks], F32, tag="sump")
        sq_parts = small.tile([P, nchunks], F32, tag="sqp")

        def lin_chunk(j):
            c0 = j * LIN_CHUNK
            cw = min(LIN_CHUNK, nw - c0)
            vw = min(max(npc - c0, 0), cw)          # valid (non-pad) columns
            pl = ps_lin.tile([P, LIN_CHUNK], F32, space="PSUM")
            nc.tensor.matmul(
                out=pl[:, 0:cw], lhsT=ws_t[:], rhs=featT[:, c0:c0 + cw],
                start=True, stop=False,
            )
            nc.tensor.matmul(
                out=pl[:, 0:cw], lhsT=wn_t[:], rhs=hnT[:, c0:c0 + cw],
                start=False, stop=True,
            )
            # rst = relu(pl + bias); partial sums on ACT
            nc.scalar.activation(
                out=rst[:, c0:c0 + cw], in_=pl[:, 0:cw], func=ACT.Relu,
                bias=bias_t[:],
            )
            if vw > 0:
                nc.vector.tensor_reduce(
                    out=sum_parts[:, j:j + 1], in_=rst[:, c0:c0 + vw],
                    axis=mybir.AxisListType.X, op=OP.add,
                )
                junk = stage.tile([P, LIN_CHUNK], F32, tag="junk")
                nc.scalar.activation(
                    out=junk[:, 0:vw], in_=rst[:, c0:c0 + vw], func=ACT.Square,
                    accum_out=sq_parts[:, j:j + 1],
                )
            else:
                nc.vector.memset(sum_parts[:, j:j + 1], 0.0)
                nc.vector.memset(sq_parts[:, j:j + 1], 0.0)

        # hoist num_idxs values into persistent registers: a fresh MOVE per
        # gather WAR-hazards on the shared register and stalls the Pool queue
        sizes = set()
        for t in range(T):
            for kn in (klo_t[t], khi_t[t]):
                for cb in range(0, kn, GMAX):
                    sizes.add(min(GMAX, kn - cb) * P)
        regs = {sz: nc.gpsimd.to_reg(sz) for sz in sorted(sizes)}

        lin_done = 0
        qrr = 0  # SWDGE queue round-robin
        for t in range(T):
            K = k_t[t]
            K_LO = klo_t[t]
            K_HI = khi_t[t]
            # ---- gathers (4 Q7 pairs in parallel via queue rotation) ----
            g = gpool.tile([P, KMAX, P], BF16, tag="g")
            for tab, k0, kn in ((flo_d, 0, K_LO), (fhi_d, K_LO, K_HI)):
                for cb in range(0, kn, GMAX):
                    cn = min(GMAX, kn - cb)
                    nc.gpsimd.dma_gather(
                        out_ap=g[:, k0 + cb:k0 + cb + cn, :],
                        in_ap=tab.ap(),
                        idxs_ap=idx_t[:, (offk[t] + k0 + cb) * 8:
                                      (offk[t] + k0 + cb + cn) * 8],
                        num_idxs=cn * P,
                        num_idxs_reg=regs[cn * P],
                        elem_size=P,
                        queue_num=qrr % NQ,
                    )
                    qrr += 1
            if t == 0:
                late_consts()
            # ---- S_w[p, c, j] = (j == dstl[p,c]) * w[p,c]  (2x DVE) ----
            s = spool.tile([P, KMAX, P], BF16)
            nc.vector.tensor_tensor(
                out=s[:, 0:K, :], in0=_pair_ap(dstl_t, t), in1=_iota_rep(t),
                op=OP.is_equal,
            )
            sw = swpool.tile([P, KMAX, P], BF16)
            nc.vector.tensor_tensor(
                out=sw[:, 0:K, :], in0=s[:, 0:K, :], in1=_pair_ap(w_t, t),
                op=OP.mult,
            )
            # ---- segment sum on PE, feature-major output ----
            # out[f, d] += sum_e g[e, f] * sw[e, d]  ->  h_neigh.T directly
            ps = ps_acc.tile([P, P], F32, space="PSUM")
            for c in range(K):
                nc.tensor.matmul(
                    out=ps[:],
                    lhsT=g[:, c, :],
                    rhs=sw[:, c, :],
                    start=(c == 0),
                    stop=(c == K - 1),
                )
            nc.scalar.activation(
                out=hnT[:, t * P:(t + 1) * P], in_=ps[:], func=ACT.Copy,
            )
            # interleave ready linear chunks (chunk j needs tiles < 4(j+1))
            while lin_done < nchunks and (lin_done + 1) * LIN_CHUNK <= (t + 1) * P:
                lin_chunk(lin_done)
                lin_done += 1

        while lin_done < nchunks:
            lin_chunk(lin_done)
            lin_done += 1

        stats = small.tile([P, 2], F32, tag="stats")
        nc.vector.tensor_reduce(
            out=stats[:, 0:1], in_=sum_parts[:, 0:nchunks],
            axis=mybir.AxisListType.X, op=OP.add
        )
        nc.vector.tensor_reduce(
            out=stats[:, 1:2], in_=sq_parts[:, 0:nchunks],
            axis=mybir.AxisListType.X, op=OP.add
        )
        nc.sync.dma_start(cc_in[:, :], stats[:])
        nc.gpsimd.collective_compute(
            "AllReduce",
            OP.add,
            replica_groups=[list(range(n_cores))],
            ins=[cc_in.ap().opt()],
            outs=[cc_out.ap().opt()],
        )
        gstats = small.tile([P, 2], F32, tag="gstats")
        nc.sync.dma_start(gstats[:], cc_out[:, :])

        # ---- BN scale/shift ----
        inv_n = 1.0 / N
        mu = small.tile([P, 1], F32, tag="mu")
        nc.vector.tensor_scalar(
            out=mu[:], in0=gstats[:, 0:1], scalar1=inv_n, scalar2=None, op0=OP.mult
        )
        var = small.tile([P, 1], F32, tag="var")
        nc.vector.tensor_scalar(
            out=var[:], in0=gstats[:, 1:2], scalar1=inv_n, scalar2=None, op0=OP.mult
        )
        mu2 = small.tile([P, 1], F32, tag="mu2")
        nc.vector.tensor_tensor(out=mu2[:], in0=mu[:], in1=mu[:], op=OP.mult)
        nc.vector.tensor_tensor(out=var[:], in0=var[:], in1=mu2[:], op=OP.subtract)
        eps_t = small.tile([P, 1], F32, tag="eps")
        nc.vector.memset(eps_t[:], EPS_BN)
        std = small.tile([P, 1], F32, tag="std")
        nc.scalar.activation(out=std[:], in_=var[:], func=ACT.Sqrt, bias=eps_t[:])
        rstd = small.tile([P, 1], F32, tag="rstd")
        nc.vector.reciprocal(rstd[:], std[:])
        scale = small.tile([P, 1], F32, tag="scale")
        nc.vector.tensor_tensor(out=scale[:], in0=gamma_t[:], in1=rstd[:], op=OP.mult)
        shift = small.tile([P, 1], F32, tag="shift")
        nc.vector.tensor_tensor(out=shift[:], in0=mu[:], in1=scale[:], op=OP.mult)
        nc.vector.tensor_tensor(out=shift[:], in0=beta_t[:], in1=shift[:], op=OP.subtract)

        # ---- apply + write out (alternate DVE / ACT) ----
        for j in range((npc + LIN_CHUNK - 1) // LIN_CHUNK):
            c0 = j * LIN_CHUNK
            cw = min(LIN_CHUNK, npc - c0)
            ot = stage.tile([P, LIN_CHUNK], F32, tag=f"ostage{j % 2}")
            nc.vector.tensor_scalar(
                out=ot[:, 0:cw], in0=rst[:, c0:c0 + cw],
                scalar1=scale[:], scalar2=shift[:], op0=OP.mult, op1=OP.add,
            )
            nc.sync.dma_start(out_d[:, c0:c0 + cw], ot[:, 0:cw])

    nc.compile()
    return nc


_cache = {}


def _get_program(key_params):
    key = tuple(sorted(key_params.items()))
    if key not in _cache:
        _cache[key] = _build_program(**key_params)
    return _cache[key]


def _in_maps(plan, W_neigh, W_self, b_self, bias, gamma, beta):
    wn_t = np.ascontiguousarray(W_neigh.T).astype(ml_dtypes.bfloat16)
    ws_t = np.ascontiguousarray(W_self.T).astype(ml_dtypes.bfloat16)
    bias_sum = (np.asarray(b_self) + np.asarray(bias)).astype(np.float32).reshape(P, 1)
    maps = []
    for c in range(N_CORES):
        maps.append({
            "feat_lo": plan["feat_lo"],
            "feat_hi": plan["feat_hi"],
            "idx_sb": plan["idx_sb"][c],
            "w_sb": plan["w_sb"][c],
            "dstl_sb": plan["dstl_sb"][c],
            "feat_self": plan["feat_self"][c],
            "iota": plan["iota"],
            "wn_t": wn_t,
            "ws_t": ws_t,
            "bias_sum": bias_sum,
            "gamma_c": np.asarray(gamma, np.float32).reshape(P, 1),
            "beta_c": np.asarray(beta, np.float32).reshape(P, 1),
        })
    return maps


def kernel(feat, src, dst, edge_weight, W_neigh, W_self, b_self, bias, gamma, beta):
    N, D = feat.shape
    plan = _host_plan(
        np.asarray(feat), np.asarray(src), np.asarray(dst), np.asarray(edge_weight)
    )
    npc = plan["npc"]

    nc = _get_program(dict(
        N=N, T=plan["T"], klo_t=plan["klo_t"], khi_t=plan["khi_t"],
        npc=npc, nw=plan["nw"], n_lo=plan["n_lo"], n_hi=plan["n_hi"],
    ))

    maps = _in_maps(plan, W_neigh, W_self, b_self, bias, gamma, beta)
    res = run_bass_kernel_spmd(nc, maps, core_ids=list(range(N_CORES)))
    out = np.empty((N, P), np.float32)
    for c in range(N_CORES):
        out[c * npc:(c + 1) * npc] = res.results[c]["outT"].T
    return out


# revision 22
# speedup vs baseline: 1.1462x; 1.1462x over previous
"""Trainium2 Bass kernel for GNN message passing (IntraConv + BatchNorm).

Computation (reference):
    msg   = feat[src] * edge_weight                    [E, D]
    neigh = segment_sum(msg, dst, N)                   [N, D]
    deg   = segment_sum(edge_weight, dst, N)           [N, 1]
    h     = relu(feat @ Ws.T + b_self + (neigh/(deg+eps)) @ Wn.T + bias)
    out   = batchnorm(h; gamma, beta)  (training-mode batch stats)

Distribution over 8 NeuronCores: edges sorted by dst and sharded by dst-range
so each core owns N/8 contiguous nodes and every edge pointing at them.
Local segment sums are exact; the only collective is an AllReduce of the
[128, 2] BatchNorm statistics.

Per-core pipeline (v7, ~3.1x over the first working version):
  - bf16 feature table (256B rows) split at 32768 for int16 dma_gather
    indices; per-128-dst-tile gathers are issued round-robin over 4
    SWDGE queues so all four Q7 core-pairs generate descriptors
    concurrently (~3.5x gather throughput; the gather is descriptor-
    generation-bound at ~2.8ns/index aggregate, not HBM-bound).
  - per-tile chunk counts (max over cores, one SPMD program) instead of
    a global max: 8.6% fewer padded gather slots.  num_idxs register
    values are hoisted into persistent registers.
  - one-hot-times-weight S_w[e,d] = (d == dstl[e]) * w~[e] built with two
    pair-trick tensor_tensor ops in bf16 2x DVE mode, where w~ folds the
    host-precomputed degree normalization 1/(deg[dst]+eps) (pure graph
    preprocessing) into the edge weight.
  - PE computes sum_e g[e,f] * S_w[e,d] with g as the stationary operand,
    so PSUM holds h_neigh.T (feature-major) directly - no transpose, no
    separate degree matmuls; ACT evacuates PSUM to the hnT slab.
  - linear chunks (512 nodes) interleave with the tile loop as their hnT
    columns complete; bias+relu and BN partial stats on ACT (Relu /
    Square with accum_out); one [128, 2] AllReduce; scale/shift; output
    written feature-major [128, N/8] and transposed on the host.
"""

import numpy as np
import ml_dtypes
from contextlib import ExitStack

import concourse.bass as bass
import concourse.tile as tile
from concourse import bacc, mybir
from concourse.bass_utils import run_bass_kernel_spmd
from concourse.masks import make_identity

N_CORES = 8
P = 128
HALF = 32768        # int16 index limit for dma_gather
LIN_CHUNK = 512
GMAX = 8            # dma_gather is limited to 1024 indices per instruction
EPS_DEG = 1e-8
EPS_BN = 1e-5

NQ = 4              # SWDGE queues: 4 Q7 core-pairs generate descriptors in parallel

F32 = mybir.dt.float32
BF16 = mybir.dt.bfloat16
I16 = mybir.dt.int16
OP = mybir.AluOpType
ACT = mybir.ActivationFunctionType


def _host_plan(feat, src, dst, edge_weight):
    N, D = feat.shape
    E = src.shape[0]
    assert D == P and N % N_CORES == 0
    npc = N // N_CORES                      # nodes per core
    T = (npc + P - 1) // P                  # dst tiles per core
    nw = T * P                              # padded node-slab width
    n_hi = N - HALF if N > HALF else 0

    src64 = src.astype(np.int64)
    dst64 = dst.astype(np.int64)
    ws_all = edge_weight.reshape(-1).astype(np.float32)

    # fold degree normalization into the edge weights (host graph prep):
    # wd_e = w_e / (deg[dst_e] + eps)
    deg_all = np.bincount(dst64, weights=ws_all.astype(np.float64), minlength=N)
    dinv_all = (1.0 / (deg_all + EPS_DEG)).astype(np.float32)
    ws_all = ws_all * dinv_all[dst64]

    half = (src64 >= HALF).astype(np.int64)
    ct = (dst64 // npc) * T + (dst64 % npc) // P      # (core, tile) group id
    order = np.lexsort((half, ct))
    ss = src64[order]
    ws = ws_all[order]
    hh = half[order]
    cts = ct[order]
    dstl = ((dst64[order] % npc) % P).astype(np.float32)

    grp = cts * 2 + hh                                 # (core, tile, half)
    counts = np.bincount(grp, minlength=N_CORES * T * 2)
    cnt_lo = counts[0::2].reshape(N_CORES, T)
    cnt_hi = counts[1::2].reshape(N_CORES, T)
    # per-tile chunk counts: max over cores (one SPMD program, 8.6% fewer
    # padded slots than a single global max)
    klo_t = np.maximum(np.ceil(cnt_lo.max(axis=0) / P).astype(np.int64), 1)
    if n_hi > 0:
        khi_t = np.maximum(np.ceil(cnt_hi.max(axis=0) / P).astype(np.int64), 1)
    else:
        khi_t = np.zeros(T, np.int64)
    k_t = klo_t + khi_t
    offk = np.zeros(T + 1, np.int64)
    np.cumsum(k_t, out=offk[1:])
    sumK = int(offk[-1])

    starts = np.zeros(N_CORES * T * 2 + 1, np.int64)
    np.cumsum(counts, out=starts[1:])
    pos = np.arange(E, dtype=np.int64) - starts[grp]
    # slot position within the core stream: tile base + (hi? lo-extent) + pos
    tix = cts % T
    q = offk[tix] * P + hh * (klo_t[tix] * P) + pos
    flat = (cts // T) * (sumK * P) + q

    # 0 index padding (the -1 trim path crashes on this runtime);
    # dstl padding 200 never matches iota (0..127); w padding 0.
    idx_stream = np.zeros(N_CORES * sumK * P, np.int32)
    w_stream = np.zeros(N_CORES * sumK * P, np.float32)
    dstl_stream = np.full(N_CORES * sumK * P, 200.0, np.float32)
    idx_stream[flat] = ss - hh * HALF
    w_stream[flat] = ws
    dstl_stream[flat] = dstl

    # stream position q = c*128 + p -> SBUF [P, sumK] at column offk[t] + c
    def to_sb(a):
        return np.ascontiguousarray(
            a.reshape(N_CORES, sumK, P).transpose(0, 2, 1)
        )

    w_sb = np.repeat(to_sb(w_stream).astype(ml_dtypes.bfloat16), 2, axis=2)
    dstl_sb = np.repeat(to_sb(dstl_stream).astype(ml_dtypes.bfloat16), 2, axis=2)

    # gather indices: [16-wrap, replicate x8] per (tile, half)
    ist = idx_stream.reshape(N_CORES, sumK, P)
    idx_cols = []
    for t in range(T):
        for h0, hn in ((offk[t], klo_t[t]), (offk[t] + klo_t[t], khi_t[t])):
            blk = ist[:, h0:h0 + hn].reshape(N_CORES, hn * P)
            blk = blk.reshape(N_CORES, hn * P // 16, 16).transpose(0, 2, 1)
            idx_cols.append(np.tile(blk, (1, 8, 1)))
    idx_sb = np.ascontiguousarray(np.concatenate(idx_cols, axis=2)).astype(np.int16)

    # bf16 gather tables (256B rows)
    feat16 = feat.astype(ml_dtypes.bfloat16)
    feat_lo = np.ascontiguousarray(feat16[:HALF])
    feat_hi = (
        np.ascontiguousarray(feat16[HALF:])
        if n_hi > 0 else np.zeros((1, P), ml_dtypes.bfloat16)
    )

    # per-core self-feature slab, bf16, zero padded to nw rows
    feat_self = np.zeros((N_CORES, nw, P), ml_dtypes.bfloat16)
    fb = feat16.reshape(N_CORES, npc, P)
    for c in range(N_CORES):
        feat_self[c, :npc] = fb[c]

    iota = np.broadcast_to(np.arange(P, dtype=np.float32), (P, P)).astype(
        ml_dtypes.bfloat16
    )

    return dict(
        N=N, E=E, npc=npc, T=T, nw=nw,
        klo_t=tuple(int(x) for x in klo_t),
        khi_t=tuple(int(x) for x in khi_t),
        sumK=sumK,
        n_lo=min(N, HALF), n_hi=max(n_hi, 1),
        idx_sb=idx_sb, w_sb=w_sb, dstl_sb=dstl_sb,
        feat_lo=feat_lo, feat_hi=feat_hi,
        feat_self=feat_self, iota=np.ascontiguousarray(iota),
    )


def _build_program(N, T, klo_t, khi_t, npc, nw, n_lo, n_hi, n_cores=N_CORES):
    k_t = [a + b for a, b in zip(klo_t, khi_t)]
    offk = [0]
    for k in k_t:
        offk.append(offk[-1] + k)
    sumK = offk[-1]
    KMAX = max(k_t)
    nc = bacc.Bacc(
        "TRN2",
        target_bir_lowering=False,
        debug=False,
        enable_asserts=False,
        num_devices=n_cores,
        num_swdge_queues=NQ,
    )

    flo_d = nc.dram_tensor("feat_lo", [n_lo, P], BF16, kind="ExternalInput")
    fhi_d = nc.dram_tensor("feat_hi", [n_hi, P], BF16, kind="ExternalInput")
    idx_d = nc.dram_tensor("idx_sb", [P, sumK * 8], I16, kind="ExternalInput")
    w_d = nc.dram_tensor("w_sb", [P, 2 * sumK], BF16, kind="ExternalInput")
    dstl_d = nc.dram_tensor("dstl_sb", [P, 2 * sumK], BF16, kind="ExternalInput")
    fself_d = nc.dram_tensor("feat_self", [nw, P], BF16, kind="ExternalInput")
    iota_d = nc.dram_tensor("iota", [P, P], BF16, kind="ExternalInput")
    wn_d = nc.dram_tensor("wn_t", [P, P], BF16, kind="ExternalInput")
    ws_d = nc.dram_tensor("ws_t", [P, P], BF16, kind="ExternalInput")
    bias_d = nc.dram_tensor("bias_sum", [P, 1], F32, kind="ExternalInput")
    gamma_d = nc.dram_tensor("gamma_c", [P, 1], F32, kind="ExternalInput")
    beta_d = nc.dram_tensor("beta_c", [P, 1], F32, kind="ExternalInput")

    out_d = nc.dram_tensor("outT", [P, npc], F32, kind="ExternalOutput")

    cc_in = nc.dram_tensor("cc_in", [P, 2], F32)
    cc_out = nc.dram_tensor("cc_out", [P, 2], F32, addr_space="Shared")

    with tile.TileContext(nc) as tc, ExitStack() as ctx:
        const = ctx.enter_context(tc.tile_pool(name="const", bufs=1))
        slabs = ctx.enter_context(tc.tile_pool(name="slabs", bufs=1))
        gpool = ctx.enter_context(tc.tile_pool(name="gpool", bufs=12))
        spool = ctx.enter_context(tc.tile_pool(name="spool", bufs=3))
        swpool = ctx.enter_context(tc.tile_pool(name="swpool", bufs=3))
        small = ctx.enter_context(tc.tile_pool(name="small", bufs=6))
        stage = ctx.enter_context(tc.tile_pool(name="stage", bufs=3))
        ps_acc = ctx.enter_context(tc.tile_pool(name="ps_acc", bufs=2, space="PSUM"))
        ps_lin = ctx.enter_context(tc.tile_pool(name="ps_lin", bufs=2, space="PSUM"))

        # ---- constants: gather-critical streams first ----
        idx_t = const.tile([P, sumK * 8], I16)
        idx_head = (offk[2] if T > 2 else sumK) * 8
        nc.sync.dma_start(idx_t[:, 0:idx_head], idx_d[:, 0:idx_head])
        nc.sync.dma_start(idx_t[:, idx_head:], idx_d[:, idx_head:])
        w_t = const.tile([P, 2 * sumK], BF16)
        nc.sync.dma_start(w_t[:], w_d[:, :])
        dstl_t = const.tile([P, 2 * sumK], BF16)
        nc.sync.dma_start(dstl_t[:], dstl_d[:, :])
        iota_t = const.tile([P, P], BF16)
        nc.sync.dma_start(iota_t[:], iota_d[:, :])
        wn_t = const.tile([P, P], BF16)
        ws_t = const.tile([P, P], BF16)
        bias_t = const.tile([P, 1], F32)
        gamma_t = const.tile([P, 1], F32)
        beta_t = const.tile([P, 1], F32)

        featT = slabs.tile([P, nw], BF16)
        rst = slabs.tile([P, nw], F32)
        hnT = slabs.tile([P, nw], BF16)

        def late_consts():
            # issued after the first tile's gathers so the gather pipeline
            # starts as early as possible
            nc.sync.dma_start(wn_t[:], wn_d[:, :])
            nc.sync.dma_start(ws_t[:], ws_d[:, :])
            nc.sync.dma_start(bias_t[:], bias_d[:, :])
            nc.sync.dma_start(gamma_t[:], gamma_d[:, :])
            nc.sync.dma_start(beta_t[:], beta_d[:, :])
            nc.sync.dma_start_transpose(featT[:], fself_d[:, :])



        def _pair_ap(tile2d, t):
            kk = k_t[t]
            a = tile2d[:, 2 * offk[t]: 2 * offk[t] + 2 * kk]
            pdim = list(a.ap)[0]
            return bass.AP(tensor=a.tensor, offset=a.offset,
                           ap=[pdim, [2, kk], [0, 64], [1, 2]])

        ia = iota_t[:]

        def _iota_rep(t):
            return bass.AP(tensor=ia.tensor, offset=ia.offset,
                           ap=[list(ia.ap)[0], [0, k_t[t]], [1, P]])

        nchunks = (nw + LIN_CHUNK - 1) // LIN_CHUNK
        sum_parts = small.tile([P, nchunks], F32, tag="sump")
        sq_parts = small.tile([P, nchunks], F32, tag="sqp")

        def lin_chunk(j):
            c0 = j * LIN_CHUNK
            cw = min(LIN_CHUNK, nw - c0)
            vw = min(max(npc - c0, 0), cw)          # valid (non-pad) columns
            pl = ps_lin.tile([P, LIN_CHUNK], F32, space="PSUM")
            nc.tensor.matmul(
                out=pl[:, 0:cw], lhsT=ws_t[:], rhs=featT[:, c0:c0 + cw],
                start=True, stop=False,
            )
            nc.tensor.matmul(
                out=pl[:, 0:cw], lhsT=wn_t[:], rhs=hnT[:, c0:c0 + cw],
                start=False, stop=True,
            )
            # rst = relu(pl + bias); partial sums on ACT
            nc.scalar.activation(
                out=rst[:, c0:c0 + cw], in_=pl[:, 0:cw], func=ACT.Relu,
                bias=bias_t[:],
            )
            if vw > 0:
                nc.vector.tensor_reduce(
                    out=sum_parts[:, j:j + 1], in_=rst[:, c0:c0 + vw],
                    axis=mybir.AxisListType.X, op=OP.add,
                )
                junk = stage.tile([P, LIN_CHUNK], F32, tag="junk")
                nc.scalar.activation(
                    out=junk[:, 0:vw], in_=rst[:, c0:c0 + vw], func=ACT.Square,
                    accum_out=sq_parts[:, j:j + 1],
                )
            else:
                nc.vector.memset(sum_parts[:, j:j + 1], 0.0)
                nc.vector.memset(sq_parts[:, j:j + 1], 0.0)

        # hoist num_idxs values into persistent registers: a fresh MOVE per
        # gather WAR-hazards on the shared register and stalls the Pool queue
        sizes = set()
        for t in range(T):
            for kn in (klo_t[t], khi_t[t]):
                for cb in range(0, kn, GMAX):
                    sizes.add(min(GMAX, kn - cb) * P)
        regs = {sz: nc.gpsimd.to_reg(sz) for sz in sorted(sizes)}

        lin_done = 0
        qrr = 0  # SWDGE queue round-robin
        for t in range(T):
            K = k_t[t]
            K_LO = klo_t[t]
            K_HI = khi_t[t]
            # ---- gathers (4 Q7 pairs in parallel via queue rotation) ----
            g = gpool.tile([P, KMAX, P], BF16, tag="g")
            for tab, k0, kn in ((flo_d, 0, K_LO), (fhi_d, K_LO, K_HI)):
                for cb in range(0, kn, GMAX):
                    cn = min(GMAX, kn - cb)
                    nc.gpsimd.dma_gather(
                        out_ap=g[:, k0 + cb:k0 + cb + cn, :],
                        in_ap=tab.ap(),
                        idxs_ap=idx_t[:, (offk[t] + k0 + cb) * 8:
                                      (offk[t] + k0 + cb + cn) * 8],
                        num_idxs=cn * P,
                        num_idxs_reg=regs[cn * P],
                        elem_size=P,
                        queue_num=qrr % NQ,
                    )
                    qrr += 1
            if t == 0:
                late_consts()
            # ---- S_w[p, c, j] = (j == dstl[p,c]) * w[p,c]  (2x DVE) ----
            s = spool.tile([P, KMAX, P], BF16)
            nc.vector.tensor_tensor(
                out=s[:, 0:K, :], in0=_pair_ap(dstl_t, t), in1=_iota_rep(t),
                op=OP.is_equal,
            )
            sw = swpool.tile([P, KMAX, P], BF16)
            nc.vector.tensor_tensor(
                out=sw[:, 0:K, :], in0=s[:, 0:K, :], in1=_pair_ap(w_t, t),
                op=OP.mult,
            )
            # ---- segment sum on PE, feature-major output ----
            # out[f, d] += sum_e g[e, f] * sw[e, d]  ->  h_neigh.T directly
            ps = ps_acc.tile([P, P], F32, space="PSUM")
            for c in range(K):
                nc.tensor.matmul(
                    out=ps[:],
                    lhsT=g[:, c, :],
                    rhs=sw[:, c, :],
                    start=(c == 0),
                    stop=(c == K - 1),
                )
            nc.scalar.activation(
                out=hnT[:, t * P:(t + 1) * P], in_=ps[:], func=ACT.Copy,
            )
            # interleave ready linear chunks (chunk j needs tiles < 4(j+1))
            while lin_done < nchunks and (lin_done + 1) * LIN_CHUNK <= (t + 1) * P:
                lin_chunk(lin_done)
                lin_done += 1

        while lin_done < nchunks:
            lin_chunk(lin_done)
            lin_done += 1

        stats = small.tile([P, 2], F32, tag="stats")
        nc.vector.tensor_reduce(
            out=stats[:, 0:1], in_=sum_parts[:, 0:nchunks],
            axis=mybir.AxisListType.X, op=OP.add
        )
        nc.vector.tensor_reduce(
            out=stats[:, 1:2], in_=sq_parts[:, 0:nchunks],
            axis=mybir.AxisListType.X, op=OP.add
        )
        nc.sync.dma_start(cc_in[:, :], stats[:])
        nc.gpsimd.collective_compute(
            "AllReduce",
            OP.add,
            replica_groups=[list(range(n_cores))],
            ins=[cc_in.ap().opt()],
            outs=[cc_out.ap().opt()],
        )
        gstats = small.tile([P, 2], F32, tag="gstats")
        nc.sync.dma_start(gstats[:], cc_out[:, :])

        # ---- BN scale/shift ----
        inv_n = 1.0 / N
        mu = small.tile([P, 1], F32, tag="mu")
        nc.vector.tensor_scalar(
            out=mu[:], in0=gstats[:, 0:1], scalar1=inv_n, scalar2=None, op0=OP.mult
        )
        var = small.tile([P, 1], F32, tag="var")
        nc.vector.tensor_scalar(
            out=var[:], in0=gstats[:, 1:2], scalar1=inv_n, scalar2=None, op0=OP.mult
        )
        mu2 = small.tile([P, 1], F32, tag="mu2")
        nc.vector.tensor_tensor(out=mu2[:], in0=mu[:], in1=mu[:], op=OP.mult)
        nc.vector.tensor_tensor(out=var[:], in0=var[:], in1=mu2[:], op=OP.subtract)
        eps_t = small.tile([P, 1], F32, tag="eps")
        nc.vector.memset(eps_t[:], EPS_BN)
        std = small.tile([P, 1], F32, tag="std")
        nc.scalar.activation(out=std[:], in_=var[:], func=ACT.Sqrt, bias=eps_t[:])
        rstd = small.tile([P, 1], F32, tag="rstd")
        nc.vector.reciprocal(rstd[:], std[:])
        scale = small.tile([P, 1], F32, tag="scale")
        nc.vector.tensor_tensor(out=scale[:], in0=gamma_t[:], in1=rstd[:], op=OP.mult)
        shift = small.tile([P, 1], F32, tag="shift")
        nc.vector.tensor_tensor(out=shift[:], in0=mu[:], in1=scale[:], op=OP.mult)
        nc.vector.tensor_tensor(out=shift[:], in0=beta_t[:], in1=shift[:], op=OP.subtract)

        # ---- apply + write out ----
        OCH = 2 * LIN_CHUNK
        for j in range((npc + OCH - 1) // OCH):
            c0 = j * OCH
            cw = min(OCH, npc - c0)
            ot = stage.tile([P, OCH], F32, tag=f"ostage{j % 2}")
            nc.vector.tensor_scalar(
                out=ot[:, 0:cw], in0=rst[:, c0:c0 + cw],
                scalar1=scale[:], scalar2=shift[:], op0=OP.mult, op1=OP.add,
            )
            nc.sync.dma_start(out_d[:, c0:c0 + cw], ot[:, 0:cw])

    nc.compile()
    return nc


_cache = {}


def _get_program(key_params):
    key = tuple(sorted(key_params.items()))
    if key not in _cache:
        _cache[key] = _build_program(**key_params)
    return _cache[key]


def _in_maps(plan, W_neigh, W_self, b_self, bias, gamma, beta):
    wn_t = np.ascontiguousarray(W_neigh.T).astype(ml_dtypes.bfloat16)
    ws_t = np.ascontiguousarray(W_self.T).astype(ml_dtypes.bfloat16)
    bias_sum = (np.asarray(b_self) + np.asarray(bias)).astype(np.float32).reshape(P, 1)
    maps = []
    for c in range(N_CORES):
        maps.append({
            "feat_lo": plan["feat_lo"],
            "feat_hi": plan["feat_hi"],
            "idx_sb": plan["idx_sb"][c],
            "w_sb": plan["w_sb"][c],
            "dstl_sb": plan["dstl_sb"][c],
            "feat_self": plan["feat_self"][c],
            "iota": plan["iota"],
            "wn_t": wn_t,
            "ws_t": ws_t,
            "bias_sum": bias_sum,
            "gamma_c": np.asarray(gamma, np.float32).reshape(P, 1),
            "beta_c": np.asarray(beta, np.float32).reshape(P, 1),
        })
    return maps


def kernel(feat, src, dst, edge_weight, W_neigh, W_self, b_self, bias, gamma, beta):
    N, D = feat.shape
    plan = _host_plan(
        np.asarray(feat), np.asarray(src), np.asarray(dst), np.asarray(edge_weight)
    )
    npc = plan["npc"]

    nc = _get_program(dict(
        N=N, T=plan["T"], klo_t=plan["klo_t"], khi_t=plan["khi_t"],
        npc=npc, nw=plan["nw"], n_lo=plan["n_lo"], n_hi=plan["n_hi"],
    ))

    maps = _in_maps(plan, W_neigh, W_self, b_self, bias, gamma, beta)
    res = run_bass_kernel_spmd(nc, maps, core_ids=list(range(N_CORES)))
    out = np.empty((N, P), np.float32)
    for c in range(N_CORES):
        out[c * npc:(c + 1) * npc] = res.results[c]["outT"].T
    return out
